# revision 26
# baseline (speedup 1.0000x reference)
# Triplane FCDecoder kernel for 8x TRN2 NeuronCores.
#
# Math: out[b,n] = sum_{pl in xz,xy,yz} bilinear(plane_pl[b], uv_pl(p[b,n])) . fc_w[:128]
#                  + p[b,n,:] . fc_w[128:131] + fc_b
# Because the decoder is linear, each plane is first projected through
# fc_w[:128] (a [1,128]x[128,HW] matmul), turning 100 MB of plane features
# into twelve 128x128 scalar tables; bilinear sampling then gathers 2x2
# corners of those tables per query point.
#
# Sharding: query points are split 8 ways (12500/batch/core). The projection
# reads each core's 1/8 column shard of all 12 (plane,batch) tables (shipped
# bf16 to halve the HBM read); an AllGather (plane 0 first, so its table can
# be built/filled while planes 1-2 still gather) replicates the projected
# tables; each core then samples only its own points.
#
# Per-plane sampling needs each partition to hold a private copy of its
# batch's 128 KB quad table (the GPSIMD gather reads partition-locally).
# Replicating via DMA partition-broadcast costs ~298 us/plane on real HW
# (HWDGE-queue-bound at ~54 GB/s for the 16 MB of SBUF writes; measured by
# slope timing). Instead, the quad rows are staged 512 KB/plane into [4, *]
# SBUF rows and replicated on the PE with a block-diagonal-ones [4,128] lhsT
# (out[p, :] = quad[p // 32]); DVE/ACT alternate evacuating PSUM into the
# table, ~40-70 us/plane, with staging and matmuls prefetched under the
# previous plane's 205 us gather.

import numpy as np

B, N, C, RES = 4, 100000, 128, 128
NCORES = 8
NP = N // NCORES            # points per batch per core (12500)
M = 391                     # slots per partition (32*391 = 12512 >= NP padded)
NPAD = 32 * M               # padded points per batch per core
J = 16 * M                  # gather indices per Q7 core per instruction
COLS = (RES * RES) // NCORES  # table column shard per core (2048)
PAD = 0.1
EPS = 1e-5

# x = clip(p/(1+PAD+EPS) + 0.5, 0, 1-EPS) * (RES-1), fused into one affine+clip.
_C1 = float(np.float32(RES - 1) / np.float32(1.0 + PAD + EPS))
_C2 = float(np.float32(0.5) * np.float32(RES - 1))
_XMAX = float(np.float32(np.float32(1.0 - EPS) * np.float32(RES - 1)))

# (ia, ib) per plane: u -> x/W axis, v -> y/H axis; pair index = plane*4 + b
_PLANES = [(0, 2), (0, 1), (1, 2)]  # xz, xy, yz

_prog_cache = {}

# test-only knob: number of extra (timing-probe) gather instructions appended
EXTRA_GATHER_REPS = 0
DEBUG_OUTPUTS = False


def _build_program():
    import concourse.bacc as bacc
    import concourse.tile as tile
    import concourse.mybir as mybir
    import concourse.bass as cbass
    from concourse.bass import _add_dep_helper

    f32 = mybir.dt.float32
    f32r = mybir.dt.float32r
    bf16 = mybir.dt.bfloat16
    i16 = mybir.dt.int16

    nc = bacc.Bacc(
        "TRN2",
        target_bir_lowering=False,
        debug=False,
        enable_asserts=False,
        num_devices=NCORES,
    )

    p_sw = nc.dram_tensor("p_sw", [128, M * 3], f32, kind="ExternalInput")
    pl_shard = nc.dram_tensor("pl_shard", [12, 128, COLS], bf16, kind="ExternalInput")
    w_pl = nc.dram_tensor("w_pl", [128, 1], bf16, kind="ExternalInput")
    # block-diagonal ones [4, 128]: bcast_w[b, p] = 1 if p // 32 == b.
    # lhsT of the PE table-broadcast (replicates batch b's quad table to
    # partitions 32b..32b+31; ~5x faster than DMA partition-broadcast).
    bcast_w = nc.dram_tensor("bcast_w", [4, 128], bf16, kind="ExternalInput")
    out_d = nc.dram_tensor("out_sw", [128, M], f32, kind="ExternalOutput")
    if DEBUG_OUTPUTS:
        dbg_quad = nc.dram_tensor("dbg_quad", [12, RES * RES, 4], mybir.dt.bfloat16, kind="ExternalOutput")
        dbg_idx = nc.dram_tensor("dbg_idx", [128, M], mybir.dt.int16, kind="ExternalOutput")
        dbg_g = nc.dram_tensor("dbg_g", [128, 16 * M, 4], mybir.dt.bfloat16, kind="ExternalOutput")
        dbg_a2 = nc.dram_tensor("dbg_a2", [128, M, 4], mybir.dt.bfloat16, kind="ExternalOutput")
        dbg_wx = nc.dram_tensor("dbg_wx", [128, M], mybir.dt.float32, kind="ExternalOutput")
        dbg_wy = nc.dram_tensor("dbg_wy", [128, M], mybir.dt.float32, kind="ExternalOutput")

    HW = RES * RES  # 16384

    with tile.TileContext(nc) as tc:
        with (
            tc.tile_pool(name="const", bufs=1) as constp,
            tc.tile_pool(name="wts", bufs=1) as wts,
            tc.tile_pool(name="tmp", bufs=1) as tmp,
            tc.tile_pool(name="psum2", bufs=2, space="PSUM") as psum2,
            tc.tile_pool(name="stage", bufs=2) as stagep,
            tc.tile_pool(name="dram", bufs=1, space="DRAM") as dram,
        ):
            # ---------------- phase 1: projection ----------------
            w_tile = constp.tile([128, 1], bf16)
            nc.sync.dma_start(w_tile[:], w_pl.ap())

            # plane-0 shards AllGather'd separately so its quad table and
            # SBUF distribute can start while planes 1-2 still gather.
            shard0_d = dram.tile([4, COLS], bf16)
            shard12_d = dram.tile([8, COLS], bf16)
            with tc.tile_pool(name="ph1", bufs=3) as ph1:
                for j in range(12):
                    stage = ph1.tile([1, COLS], bf16, tag="stage")
                    # pl_shard[j] is fully contiguous (128 rows x 4 KB):
                    # one fat linear DMA per table, alternating both HWDGE
                    # queues, instead of 4 strided 1KB-run chunk loads on
                    # one queue (strided/small-descriptor DMAs measured
                    # heavily queue-bound).
                    chunk = ph1.tile([128, COLS], bf16, tag="chunk")
                    eng = nc.sync if (j % 2 == 0) else nc.scalar
                    eng.dma_start(chunk[:], pl_shard.ap()[j])
                    for k in range(COLS // 512):
                        pt = psum2.tile([128, 2048], f32, tag="p2")
                        nc.tensor.matmul(
                            pt[0:1, 0:512], lhsT=w_tile[:],
                            rhs=chunk[:, 512 * k : 512 * (k + 1)],
                            start=True, stop=True,
                        )
                        nc.scalar.copy(stage[0:1, 512 * k : 512 * (k + 1)],
                                       pt[0:1, 0:512])
                    if j < 4:
                        nc.scalar.dma_start(shard0_d[j : j + 1], stage[:])
                    else:
                        nc.scalar.dma_start(shard12_d[j - 4 : j - 3], stage[:])

            # ---------------- phase 2: allgather raw tables ----------------
            ag0 = dram.tile([NCORES, 4, COLS], bf16)
            ag12 = dram.tile([NCORES, 8, COLS], bf16)
            nc.gpsimd.collective_compute(
                "AllGather",
                mybir.AluOpType.bypass,
                replica_groups=[list(range(NCORES))],
                ins=[shard0_d.opt()],
                outs=[ag0.opt()],
            )
            nc.gpsimd.collective_compute(
                "AllGather",
                mybir.AluOpType.bypass,
                replica_groups=[list(range(NCORES))],
                ins=[shard12_d.opt()],
                outs=[ag12.opt()],
            )

            # ---------------- phase 2b: quad table construction -------------
            # quad_d[j, s] = (T[s], T[s+1], T[s+128], T[s+129]); entries whose
            # window crosses a table edge are garbage but never sampled
            # (s = 128*y0 + x0 with x0<=126, y0<=126).
            # Global element g = j*HW + r*COLS + c lives at ag_out[r, j, c].
            # Partition p <-> (j, r) = (p//8, p%8) holds chunk [2048] + 130
            # overlap.
            quad_d = dram.tile([12, HW, 4], bf16)
            with tc.tile_pool(name="quadc", bufs=1) as qc:
                tshq = qc.tile([96, COLS + 130], bf16)
                # r == 7 overlap columns cover s >= 16255, never sampled
                # (x0, y0 <= 126 so s <= 16254): zeros instead of a fetch.
                nc.vector.memset(tshq[:, COLS : COLS + 130], 0.0)
                qd = quad_d[:]
                quadseg = qc.tile([96, COLS, 4], bf16)
                # group 0 = plane 0 (tables 0-3, partitions 0:32, fed by ag0);
                # group 1 = planes 1-2 (tables 4-11, partitions 32:96, ag12).
                for agt, njt, p0 in ((ag0[:], 4, 0), (ag12[:], 8, 32)):
                    # main chunk load split across both HWDGE queues (these
                    # are 4KB-run strided DMAs, which are queue-bound)
                    nh = njt // 2
                    for h in range(2):
                        src_main = cbass.AP(
                            tensor=agt.tensor,
                            offset=agt.offset + h * nh * COLS,
                            ap=[[COLS, nh], [njt * COLS, NCORES], [1, COLS]],
                        )
                        heng = nc.sync if h == 0 else nc.scalar
                        heng.dma_start(
                            tshq[p0 + 8 * nh * h : p0 + 8 * nh * (h + 1), 0:COLS],
                            src_main,
                        )
                    # overlap elems: partition (j, r) needs the next chunk's
                    # first 130 elems: r < 7 -> ag[r+1, j, 0:130].
                    for r in range(7):
                        src_b = cbass.AP(
                            tensor=agt.tensor,
                            offset=agt.offset + (r + 1) * njt * COLS,
                            ap=[[COLS, njt], [1, 130]],
                        )
                        reng = nc.scalar if (r % 2 == 0) else nc.sync
                        reng.dma_start(
                            tshq[p0 + r : p0 + 8 * njt : 8, COLS : COLS + 130],
                            src_b,
                        )
                    for ci, off in enumerate((0, 1, 128, 129)):
                        # DVE APs starting at partition >= 32 may span at
                        # most 32 partitions: chunk the copy.
                        for q0 in range(p0, p0 + 8 * njt, 32):
                            nc.vector.tensor_copy(
                                quadseg[q0 : q0 + 32, :, ci],
                                tshq[q0 : q0 + 32, off : off + COLS],
                            )
                    dst_q = cbass.AP(
                        tensor=qd.tensor,
                        offset=qd.offset + p0 * (COLS * 4),
                        ap=[[COLS * 4, 8 * njt], [1, COLS * 4]],
                    )
                    nc.sync.dma_start(dst_q, quadseg[p0 : p0 + 8 * njt])

            # ---------------- phase 3/4: distribute + sample per plane ------
            tabgath = tc.tile_pool(name="tables", bufs=1)
            tabp = tabgath.__enter__()
            gath_cm = tc.tile_pool(name="gath", bufs=1)
            gathp = gath_cm.__enter__()
            tab = tabp.tile([128, HW, 4], bf16)

            bw = constp.tile([4, 128], bf16)
            nc.sync.dma_start(bw[:], bcast_w.ap())

            # PE table broadcast: stage the plane's 4 quad tables (512 KB)
            # from DRAM into [4, *] SBUF rows, then matmul with the
            # block-diagonal ones lhsT to replicate them across the 128
            # partitions of tab; DVE/ACT evacuate PSUM -> tab (bf16).
            # Only the evacuations write tab, so staging + matmuls prefetch
            # under the previous plane's gather.
            STG = 1024           # bf16 elems staged per batch-row per chunk
            PW = 2048            # f32 elems per PSUM evac tile (2 chunks)
            def distribute(plane, after=None):
                insts = []
                p2 = None
                for c8 in range(HW * 4 // STG):  # 64 staging chunks
                    st = stagep.tile([4, STG], bf16, tag="st")
                    src = cbass.AP(
                        tensor=qd.tensor,
                        offset=qd.offset + plane * 4 * (HW * 4) + c8 * STG,
                        ap=[[HW * 4, 4], [1, STG]],
                    )
                    eng = nc.sync if (c8 % 2 == 0) else nc.scalar
                    eng.dma_start(st[:], src)
                    if c8 % 2 == 0:
                        p2 = psum2.tile([128, PW], f32, tag="p2")
                    half = (c8 % 2) * STG  # f32 offset within p2 (STG per chunk)
                    for cm in range(STG // 512):  # 2 matmuls per chunk
                        nc.tensor.matmul(
                            p2[:, half + cm * 512 : half + (cm + 1) * 512],
                            lhsT=bw[:],
                            rhs=st[:, cm * 512 : (cm + 1) * 512],
                            start=True,
                            stop=True,
                        )
                    if c8 % 2 == 1:
                        q0 = ((c8 - 1) * STG) // 4  # quad index in tab
                        # alternate evacuations across DVE and ACT
                        veng = nc.vector if ((c8 // 2) % 2 == 0) else nc.scalar
                        if veng is nc.vector:
                            di = veng.tensor_copy(
                                tab[:, q0 : q0 + PW // 4, :], p2[:]
                            )
                        else:
                            di = veng.copy(tab[:, q0 : q0 + PW // 4, :], p2[:])
                        if after is not None:
                            _add_dep_helper(
                                di.ins, after.ins, True,
                                "table rewrite waits for prev plane gather")
                        insts.append(di)
                return insts

            p_sb = constp.tile([128, M, 3], f32)
            nc.sync.dma_start(p_sb[:], p_sw.ap())

            acc = constp.tile([128, M], f32)

            prev_gather = None
            prev_dediag = []
            for pli, (ia, ib) in enumerate(_PLANES):
                dist_insts = distribute(pli, after=prev_gather)

                # weights / indices, all M slots at once
                xt = tmp.tile([128, M], f32, tag="sc0")
                nc.vector.tensor_scalar(
                    xt[:], p_sb[:, :, ia], _C1, _C2,
                    mybir.AluOpType.mult, mybir.AluOpType.add,
                )
                nc.vector.tensor_scalar(
                    xt[:], xt[:], 0.0, _XMAX,
                    mybir.AluOpType.max, mybir.AluOpType.min,
                )
                # floor(x) for x>=0, robust to trunc or round-nearest casts:
                # xi = int(x); x0 = xi - (xi > x); wx = x - x0
                xi = tmp.tile([128, M], mybir.dt.int32, tag="xi")
                nc.vector.tensor_copy(xi[:], xt[:])
                xf = tmp.tile([128, M], f32, tag="xf")
                nc.vector.tensor_copy(xf[:], xi[:])
                mk = tmp.tile([128, M], f32, tag="mk")
                nc.vector.tensor_tensor(mk[:], xf[:], xt[:], mybir.AluOpType.is_gt)
                x0 = tmp.tile([128, M], f32, tag="sc1")
                nc.vector.tensor_tensor(x0[:], xf[:], mk[:], mybir.AluOpType.subtract)
                wx = wts.tile([128, M], bf16, tag="wx")
                nc.vector.tensor_tensor(wx[:], xt[:], x0[:], mybir.AluOpType.subtract)

                yt = tmp.tile([128, M], f32, tag="sc2")
                nc.scalar.activation(
                    yt[:], p_sb[:, :, ib], mybir.ActivationFunctionType.Copy,
                    bias=_C2, scale=_C1,
                )
                nc.vector.tensor_scalar(
                    yt[:], yt[:], 0.0, _XMAX,
                    mybir.AluOpType.max, mybir.AluOpType.min,
                )
                nc.vector.tensor_copy(xi[:], yt[:])
                nc.vector.tensor_copy(xf[:], xi[:])
                nc.vector.tensor_tensor(mk[:], xf[:], yt[:], mybir.AluOpType.is_gt)
                y0 = tmp.tile([128, M], f32, tag="sc3")
                nc.vector.tensor_tensor(y0[:], xf[:], mk[:], mybir.AluOpType.subtract)
                wy = wts.tile([128, M], bf16, tag="wy")
                nc.vector.tensor_tensor(wy[:], yt[:], y0[:], mybir.AluOpType.subtract)
                # s = y0*128 + x0 (reuses the yt slot; yt is dead here)
                st = tmp.tile([128, M], f32, tag="sc2")
                nc.vector.tensor_scalar(
                    st[:], y0[:], float(RES), None, mybir.AluOpType.mult
                )
                nc.vector.tensor_tensor(st[:], st[:], x0[:], mybir.AluOpType.add)

                idx0 = wts.tile([128, M], i16, tag="idx0")
                nc.vector.tensor_copy(idx0[:], st[:])

                # one gather per plane: all four corners per index
                g2 = gathp.tile([128, J, 4], bf16, tag="g2")
                gi = nc.gpsimd.ap_gather(
                    g2[:], tab[:], idx0[:],
                    channels=128, num_elems=HW, d=4, num_idxs=J,
                )
                for di in dist_insts:
                    _add_dep_helper(gi.ins, di.ins, True, "gather waits for tables")
                for dd in prev_dediag:
                    _add_dep_helper(gi.ins, dd.ins, True, "gather waits for prev dediag")
                prev_gather = gi

                if DEBUG_OUTPUTS and pli == 0:
                    nc.sync.dma_start(dbg_idx.ap(), idx0[:])
                    nc.sync.dma_start(dbg_g.ap(), g2[:])
                # de-diagonalize: a2[p, m, :] = g2[p, 16m + p%16, :]
                a2 = tmp.tile([128, M, 4], bf16, tag="a2")
                prev_dediag = []
                for r in range(16):
                    eng = nc.sync if (r % 2 == 0) else nc.scalar
                    dd = eng.dma_start(a2[r::16], g2[r::16, r::16, :])
                    _add_dep_helper(dd.ins, gi.ins, True, "dediag waits for gather")
                    prev_dediag.append(dd)

                if DEBUG_OUTPUTS and pli == 0:
                    nc.sync.dma_start(dbg_a2.ap(), a2[:])
                    nc.sync.dma_start(dbg_wx.ap(), wx[:])
                    nc.sync.dma_start(dbg_wy.ap(), wy[:])
                # combine (f32): l0/l1 row lerps, then y lerp
                d0 = tmp.tile([128, M], f32, tag="c0")
                nc.vector.tensor_tensor(d0[:], a2[:, :, 1], a2[:, :, 0], mybir.AluOpType.subtract)
                nc.vector.tensor_tensor(d0[:], d0[:], wx[:], mybir.AluOpType.mult)
                nc.vector.tensor_tensor(d0[:], d0[:], a2[:, :, 0], mybir.AluOpType.add)

                d1 = tmp.tile([128, M], f32, tag="c1")
                nc.vector.tensor_tensor(d1[:], a2[:, :, 3], a2[:, :, 2], mybir.AluOpType.subtract)
                nc.vector.tensor_tensor(d1[:], d1[:], wx[:], mybir.AluOpType.mult)
                nc.vector.tensor_tensor(d1[:], d1[:], a2[:, :, 2], mybir.AluOpType.add)

                nc.vector.tensor_tensor(d1[:], d1[:], d0[:], mybir.AluOpType.subtract)
                nc.vector.tensor_tensor(d1[:], d1[:], wy[:], mybir.AluOpType.mult)
                nc.vector.tensor_tensor(d1[:], d1[:], d0[:], mybir.AluOpType.add)
                if pli == 0:
                    nc.vector.tensor_copy(acc[:], d1[:])
                else:
                    nc.vector.tensor_tensor(acc[:], acc[:], d1[:], mybir.AluOpType.add)

            for _rep in range(EXTRA_GATHER_REPS):
                g2x = gathp.tile([128, J, 4], bf16, tag="g2")
                nc.gpsimd.ap_gather(
                    g2x[:], tab[:], idx0[:],
                    channels=128, num_elems=HW, d=4, num_idxs=J,
                )

            nc.sync.dma_start(out_d.ap(), acc[:])
            gath_cm.__exit__(None, None, None)
            tabgath.__exit__(None, None, None)

    nc.compile()
    return nc


def _get_program():
    if "nc" not in _prog_cache:
        _prog_cache["nc"] = _build_program()
    return _prog_cache["nc"]


def kernel(p, c_xz, c_xy, c_yz, fc_w, fc_b, trace=False):
    import ml_dtypes
    from concourse import bass_utils

    nc = _get_program()

    p = np.asarray(p, dtype=np.float32)
    fc_w = np.asarray(fc_w, dtype=np.float32)
    fc_b = np.asarray(fc_b, dtype=np.float32)

    planes12 = np.empty((12, 128, RES * RES), dtype=ml_dtypes.bfloat16)
    for pli, c in enumerate([c_xz, c_xy, c_yz]):
        c = np.asarray(c, dtype=np.float32)
        planes12[pli * 4 : pli * 4 + 4] = c.reshape(B, C, RES * RES)

    w_pl_np = np.ascontiguousarray(
        fc_w[:128].reshape(128, 1).astype(ml_dtypes.bfloat16)
    )
    bcast_w_np = np.repeat(
        np.eye(4, dtype=np.float32), 32, axis=1
    ).astype(ml_dtypes.bfloat16)

    in_maps = []
    for r in range(NCORES):
        p_r = p[:, r * NP : (r + 1) * NP, :]
        p_pad = np.zeros((B, NPAD, 3), dtype=np.float32)
        p_pad[:, :NP] = p_r
        p_swz = np.ascontiguousarray(
            p_pad.reshape(B, M, 32, 3).transpose(0, 2, 1, 3).reshape(128, M * 3)
        )
        in_maps.append(
            {
                "p_sw": p_swz,
                "pl_shard": np.ascontiguousarray(
                    planes12[:, :, r * COLS : (r + 1) * COLS]
                ),
                "w_pl": w_pl_np,
                "bcast_w": bcast_w_np,
            }
        )

    res = bass_utils.run_bass_kernel_spmd(
        nc, in_maps, core_ids=list(range(NCORES)), trace=trace
    )
    if trace:
        print("exec_time_ns:", res.exec_time_ns)
        kernel.last_results = res

    out = np.empty((B, N), dtype=np.float32)
    for r in range(NCORES):
        o = res.results[r]["out_sw"]
        o = o.reshape(B, 32, M).transpose(0, 2, 1).reshape(B, NPAD)
        out[:, r * NP : (r + 1) * NP] = o[:, :NP]

    out += p @ fc_w[128:131, 0] + fc_b[0]
    return out

